# revision 41
# baseline (speedup 1.0000x reference)
"""Causal multi-head attention on 8 Trainium2 NeuronCores.

Problem (hardcoded): x [4, 2048, 1024] fp32, W_qkv [1024, 3072], b_qkv [3072],
W_o [1024, 1024], b_o [1024]; 16 heads, head_dim 64.

Sharding: 8 cores = 4 batches x 2 head-groups (8 heads each). Each core
computes QKV projection for its (batch, head-group), causal attention for its
8 heads, and a partial out-projection [2048, 1024]. Host sums the two
head-group partials per batch and adds b_o.

Kernel strategy (per core, "transposed" domain):
  - x strip [512, 1024] -> PE-transpose -> xT [128, 8ds, 512]
  - QT/KTz = W^T x^T via matmul(lhsT=W_tile, rhs=xT); KTz zero-padded per
    head so the score matmul contracts K=128 (keeps PE at full clock).
  - V natural = matmul(lhsT=xT_tile, rhs=Wv), stored [128, blk, head, 65]
    with a ones column (denominator accumulates in psO row 64).
  - Scores per (head, strip): sk-blocks processed in PAIRS sharing one
    [128,1024] 2-bank psum tile; ONE Exp activation per off-diagonal pair.
    Diagonal blocks are causally trimmed (A/exp/AV restricted to sq>=128j)
    and masked via one strided 2-corner multiply with a [128,128] triangle.
  - Normalize: denominator copy + reciprocal_approx_fast + gpsimd
    partition_broadcast; OT = psO * recip (DVE).
  - out partial = matmul(lhsT=OT tile, rhs=W_o tiles) -> [s, e] -> DMA out.
  - Software pipelining: transposes/QKV-proj of strip i+1 and out-proj of
    strip i-1 are interleaved as PE fillers between attention pairs, so the
    PE fills gaps while Scalar (Exp) paces the attention inner loop.
Projection/out-proj matmuls run float32r; attention matmuls run bf16.
"""

import ml_dtypes
import numpy as np

import concourse.bass as bass
from concourse import bacc
import concourse.mybir as mybir
from concourse.bass_utils import run_bass_kernel_spmd
from concourse.tile import TileContext

B, S, D = 4, 2048, 1024
H, HD = 16, 64
G = 2                  # head groups (cores per batch)
HPG = H // G           # 8 heads per core
NG = HPG * HD          # 512 qkv feature columns per core
N_CORES = 8
STRIP = 512            # sq strip width
NSTRIP = S // STRIP    # 4
DS = D // 128          # 8 contraction subtiles for the projections
FP32 = mybir.dt.float32
R32 = mybir.dt.float32r
BF16 = mybir.dt.bfloat16
AF = mybir.ActivationFunctionType


def build_bass(dbg=False):
    nc = bacc.Bacc("TRN2")

    xt_d = nc.dram_tensor("xt", [D, S], BF16, kind="ExternalInput")
    wq_d = nc.dram_tensor("wq", [4, 128, DS, 128], BF16, kind="ExternalInput")
    wk_d = nc.dram_tensor("wk", [4, 128, DS, 128], BF16, kind="ExternalInput")
    wv_d = nc.dram_tensor("wv", [128, DS, NG], BF16, kind="ExternalInput")
    bqk_d = nc.dram_tensor("bqk", [128, 8], FP32, kind="ExternalInput")
    tri_d = nc.dram_tensor("tri", [128, 2, 128], BF16, kind="ExternalInput")
    bv_d = nc.dram_tensor("bv", [1, NG], FP32, kind="ExternalInput")
    wo_d = nc.dram_tensor("wo", [128, 4, D], BF16, kind="ExternalInput")
    out_d = nc.dram_tensor("out", [S, D], BF16, kind="ExternalOutput")
    # ns0-1 half-contraction partial of the LAST strip's out-projection,
    # computed mid-attention (hidden) and summed on the host; leaves only
    # the ns2-3 half + evac for the post-attention tail
    out2_d = nc.dram_tensor("out2", [STRIP, D], BF16, kind="ExternalOutput")

    with TileContext(nc) as tc:
        with (
            tc.tile_pool(name="const", bufs=1) as const,
            tc.tile_pool(name="persist", bufs=1) as persist,
            tc.tile_pool(name="work", bufs=2) as work,
            tc.tile_pool(name="psum", bufs=2, space="PSUM") as psum,
        ):
            xT = {}      # strip -> list of per-ds tiles

            def emit_xT_dmas(i):
                # x arrives pre-transposed from the host; one tile per ds so
                # the first projection matmul only waits for its own slice
                s0 = i * STRIP
                xT[i] = []
                for ds in range(DS):
                    t = work.tile([128, STRIP], BF16, name=f"xT{ds}",
                                  tag=f"xT{ds}", bufs=3)
                    nc.sync.dma_start(
                        t, xt_d[ds * 128:(ds + 1) * 128, s0:s0 + STRIP])
                    xT[i].append(t)

            # Startup DMA bandwidth (~200 GB/s aggregate) is the scarce
            # resource: issue transfers strictly in consumption order so the
            # critical ones aren't starved. Strip 1 prefetches in the
            # prologue too (xT bufs=3) so the strip-0 attention's projection
            # fillers aren't data-starved.
            emit_xT_dmas(0)
            bqk_sb = const.tile([128, 8], FP32, name="bqk_sb")
            nc.sync.dma_start(bqk_sb, bqk_d[:, :])
            bv_sb = const.tile([1, NG], FP32, name="bv_sb")
            nc.sync.dma_start(bv_sb, bv_d[:, :])
            emit_xT_dmas(1)
            tri2 = const.tile([128, 2, 128], BF16, name="tri2")
            nc.sync.dma_start(tri2, tri_d[:, :, :])
            wq_sb = [const.tile([128, DS, 128], BF16, name=f"wq_sb{q}")
                     for q in range(4)]
            wk_sb = [const.tile([128, DS, 128], BF16, name=f"wk_sb{q}")
                     for q in range(4)]
            wv_sb = const.tile([128, DS, NG], BF16, name="wv_sb")
            wo_sb = const.tile([128, 4, D], BF16, name="wo_sb")
            # wq/wk stream as interleaved halves on the Activation HWDGE
            # queue so the first K-projection unblocks early; wo trails
            for q in (0, 1):
                nc.scalar.dma_start(wq_sb[q], wq_d[q])
                nc.scalar.dma_start(wk_sb[q], wk_d[q])
            nc.scalar.dma_start(wv_sb, wv_d[:, :, :])
            for q in (2, 3):
                nc.scalar.dma_start(wq_sb[q], wq_d[q])
                nc.scalar.dma_start(wk_sb[q], wk_d[q])
            # wo isn't consumed until the attention(1) out-proj fillers
            # (~60us): park it on the sync queue BEHIND the strip-1 x tiles
            # so it doesn't steal startup bandwidth from them
            nc.sync.dma_start(wo_sb, wo_d[:, :, :])
            # bias broadcast for the Vn evacuation add (one-time, on gpsimd)
            bvb = const.tile([128, NG], FP32, name="bvb")
            nc.gpsimd.partition_broadcast(bvb, bv_sb[0:1, :])

            # Persistent zero-padded K^T per head and V tiles (both bf16)
            KTz = persist.tile([128, HPG, S], BF16, name="KTz")
            # even heads occupy rows 0-63 (zero 64-127); odd heads vice versa
            for h in range(HPG):
                zrow = 64 if h % 2 == 0 else 0
                nc.gpsimd.memset(KTz[zrow:zrow + 64, h, :], 0.0)
            Vn = persist.tile([128, S // 128, HPG, HD + 1], BF16, name="Vn")
            nc.gpsimd.memset(Vn[:, :, :, HD], 1.0)

            QT = {}      # strip -> tile
            OT = {}      # strip -> tile

            def qk_chunk(i, which, nb):
                # 8 matmuls (full D contraction) + bias-add evacuation
                s0 = i * STRIP
                if which == 0 and nb == 0:
                    QT[i] = work.tile([128, 4, STRIP], BF16, name="QT",
                                      tag="QT", bufs=2)
                w_sb = (wq_sb if which == 0 else wk_sb)[nb]
                ps = psum.tile([128, STRIP], FP32, name="ps", tag="ps_mm",
                               bufs=2)
                for ds in range(DS):
                    nc.tensor.matmul(
                        ps, lhsT=w_sb[:, ds, :],
                        rhs=xT[i][ds],
                        start=(ds == 0), stop=(ds == DS - 1))
                bcol = bqk_sb[:, 4 * which + nb:4 * which + nb + 1]
                if which == 0:
                    nc.vector.tensor_scalar_add(QT[i][:, nb, :], ps, bcol)
                else:
                    nc.vector.tensor_scalar_add(
                        KTz[0:64, 2 * nb, s0:s0 + STRIP],
                        ps[0:64, :], bcol[0:64, :])
                    nc.vector.tensor_scalar_add(
                        KTz[64:128, 2 * nb + 1, s0:s0 + STRIP],
                        ps[64:128, :], bcol[64:128, :])

            def v_chunk(i, st):
                stg = i * 4 + st
                ps = psum.tile([128, STRIP], FP32, name="psv", tag="ps_mm",
                               bufs=2)
                for ds in range(DS):
                    nc.tensor.matmul(
                        ps,
                        lhsT=xT[i][ds][:, st * 128:(st + 1) * 128],
                        rhs=wv_sb[:, ds],
                        start=(ds == 0), stop=(ds == DS - 1))
                nc.vector.tensor_add(
                    Vn[:, stg, :, 0:HD],
                    ps.rearrange("p (h d) -> p h d", d=HD),
                    bvb.rearrange("p (h d) -> p h d", d=HD))

            ops_open = {}  # (i, st, ec) -> psum tile across half-chunks

            def outproj_chunk(i, st, ec, nh, evac="v", ptag="ps_mm"):
                # half-chunk: ns 0-1 (nh=0) opens the psum group, ns 2-3
                # (nh=1) closes it, evacuates, and DMAs its half out.
                # One [128,512] ob tile per (st,ec) half (bufs=4) so a
                # block's evacuation never WAR-waits on the previous
                # block's 512KB out-DMA.
                s0 = i * STRIP
                if nh == 0:
                    ps = psum.tile([128, STRIP], FP32, name="pso",
                                   tag=ptag, bufs=2)
                    ops_open[(i, st, ec)] = ps
                else:
                    ps = ops_open.pop((i, st, ec))
                for ns in (2 * nh, 2 * nh + 1):
                    nc.tensor.matmul(
                        ps,
                        lhsT=OT[i][:, ns, st * 128:(st + 1) * 128],
                        rhs=wo_sb[:, ns, ec * 512:(ec + 1) * 512],
                        start=(ns == 0), stop=(ns == 3))
                if nh == 1:
                    obt = work.tile([128, STRIP], BF16, name="ob", tag="ob",
                                    bufs=4)
                    # DMA trigger on gpsimd keeps the (busy) Scalar queue
                    # free of DIRECT2D descriptor writes
                    if evac == "s":
                        nc.scalar.copy(obt, ps)
                        trig = nc.scalar
                    else:
                        nc.vector.tensor_copy(obt, ps)
                        trig = nc.gpsimd
                    trig.dma_start(
                        out_d[s0 + st * 128:s0 + (st + 1) * 128,
                              ec * 512:(ec + 1) * 512], obt)

            def attention(i, fillers):
                def fill():
                    try:
                        next(fillers)()
                    except StopIteration:
                        pass

                npair = 2 * i + 2  # 2i off-diagonal pairs + 2 diagonal pairs
                OT[i] = work.tile([128, 4, STRIP], BF16, name="OT", tag="OT",
                                  bufs=3)
                for h in range(HPG):
                    prow = (h % 2) * 64
                    nsub = h // 2
                    psO = psum.tile([128, STRIP], FP32, name="psO", tag="psO",
                                    bufs=2)

                    def emit_avs(pend):
                        # AVs for an already-exp'd pair (one-pair lookahead:
                        # by now the exp/corner-mul are long done, so these
                        # issue without stalling the PE)
                        p, expP = pend
                        for l in range(2):
                            if p < 2 * i:
                                b, c0, j = 2 * p + l, 512 * l, 0
                            else:
                                j = 2 * (p - 2 * i) + l
                                b, c0 = 4 * i + j, 512 * l + 128 * j
                            nc.tensor.matmul(
                                psO[0:HD + 1, 128 * j:STRIP],
                                lhsT=Vn[:, b, h, :],
                                rhs=expP[:, c0:512 * (l + 1)],
                                start=(i == 0 and p == 0 and l == 0)
                                or (i > 0 and p == 0 and l == 0),
                                stop=(p == npair - 1 and l == 1),
                                skip_group_check=True)

                    pending = None
                    for p in range(npair):
                        psA = psum.tile([128, 1024], FP32, name="psA",
                                        tag="psA2", bufs=2)
                        expP = work.tile([128, 1024], BF16, name="expP",
                                         tag="expP", bufs=6)
                        if p < 2 * i:        # off-diagonal pair, full width
                            for l in range(2):
                                b = 2 * p + l
                                nc.tensor.matmul(
                                    psA[:, 512 * l:512 * (l + 1)],
                                    lhsT=KTz[:, h, b * 128:(b + 1) * 128],
                                    rhs=QT[i][:, nsub, :],
                                    start=True, stop=True)
                            nc.scalar.activation(expP, psA, AF.Exp,
                                                 scale=0.125)
                        else:                # diagonal pair, causally trimmed
                            pd = p - 2 * i
                            for l in range(2):
                                j = 2 * pd + l
                                b = 4 * i + j
                                c0 = 512 * l + 128 * j
                                nc.tensor.matmul(
                                    psA[:, c0:512 * (l + 1)],
                                    lhsT=KTz[:, h, b * 128:(b + 1) * 128],
                                    rhs=QT[i][:, nsub, 128 * j:STRIP],
                                    start=True, stop=True,
                                    skip_group_check=True)
                            # per-slot exps: skip the stale psum gap
                            # [512, 640+256pd) between the two trimmed slots
                            nc.scalar.activation(
                                expP[:, 256 * pd:512],
                                psA[:, 256 * pd:512],
                                AF.Exp, scale=0.125)
                            nc.scalar.activation(
                                expP[:, 640 + 256 * pd:1024],
                                psA[:, 640 + 256 * pd:1024],
                                AF.Exp, scale=0.125)
                            # strided 2-corner causal mask multiply
                            cbase = expP[:, 256 * pd:256 * pd + 768]
                            cap = bass.AP(
                                tensor=cbase.tensor, offset=cbase.offset,
                                ap=[list(cbase.ap[0])] + [[640, 2], [1, 128]])
                            nc.vector.tensor_mul(cap, cap, tri2)
                        if pending is not None:
                            emit_avs(pending)
                        pending = (p, expP)
                        fill()
                    emit_avs(pending)
                    # normalize: recip of denominator row, broadcast, mult.
                    # The very last head's den copy rides Scalar (idle after
                    # the final exp) to shorten the exposed tail chain.
                    den = work.tile([1, STRIP], FP32, name="den", tag="den",
                                    bufs=1)
                    if i == NSTRIP - 1 and h == HPG - 1:
                        nc.scalar.copy(den, psO[HD:HD + 1, :])
                    else:
                        nc.vector.tensor_copy(den, psO[HD:HD + 1, :])
                    recip = work.tile([1, STRIP], FP32, name="recip",
                                      tag="recip", bufs=1)
                    nc.vector.reciprocal_approx_fast(recip, den)
                    pbt = work.tile([64, STRIP], FP32, name="pbt", tag="pbt",
                                    bufs=2)
                    nc.gpsimd.partition_broadcast(pbt, recip[0:1, :])
                    nc.vector.tensor_mul(OT[i][prow:prow + 64, nsub, :],
                                         psO[0:HD, :], pbt)
                    fill()

            # ---- prologue: strip 0 projections, ordered to match DMA
            # arrival (half-0 weights, then wv, then half-1) ----
            for which in range(2):
                for nb in range(2):
                    qk_chunk(0, which, nb)
            for st in range(4):
                v_chunk(0, st)
            for which in range(2):
                for nb in range(2, 4):
                    qk_chunk(0, which, nb)

            def outproj3_half(st, ec, lo, ptag="ps_mm", evac="v"):
                # half-contraction out-proj for the final strip: lo=True does
                # ns 0-1 -> out2 partial (hidden mid-attention), lo=False does
                # ns 2-3 -> out (the only post-attention PE work)
                i = NSTRIP - 1
                nss = (0, 1) if lo else (2, 3)
                ps = psum.tile([128, STRIP], FP32, name="ps3", tag=ptag,
                               bufs=2)
                for ns in nss:
                    nc.tensor.matmul(
                        ps,
                        lhsT=OT[i][:, ns, st * 128:(st + 1) * 128],
                        rhs=wo_sb[:, ns, ec * 512:(ec + 1) * 512],
                        start=(ns == nss[0]), stop=(ns == nss[1]))
                obt = work.tile([128, STRIP], BF16, name="ob", tag="ob",
                                bufs=4)
                if evac == "s":
                    nc.scalar.copy(obt, ps)
                    trig = nc.scalar
                else:
                    nc.vector.tensor_copy(obt, ps)
                    trig = nc.gpsimd
                if lo:
                    dst = out2_d[st * 128:(st + 1) * 128,
                                 ec * 512:(ec + 1) * 512]
                else:
                    s0 = i * STRIP
                    dst = out_d[s0 + st * 128:s0 + (st + 1) * 128,
                                ec * 512:(ec + 1) * 512]
                trig.dma_start(dst, obt)

            # ---- main loop: attention(i) with interleaved fillers ----
            # v chunks of strip i are deferred INTO attention(i): the first
            # diagonal AV that reads Vn block 4i+j is emitted >= 2 fill
            # slots after the head starts (one-pair lookahead), so v(i)
            # popped in the first 4 slots always lands in time. This keeps
            # the late, filler-starved strips supplied with PE work.
            deferred_v = []
            for i in range(NSTRIP):
                front = list(deferred_v)   # MUST be the first pops (Vn deps)
                fillers = []
                deferred_v = []
                if i + 2 < NSTRIP:
                    emit_xT_dmas(i + 2)  # strips 0/1 prefetched in prologue
                if i + 1 < NSTRIP:
                    for which in range(2):
                        for nb in range(4):
                            fillers.append(
                                lambda which=which, nb=nb:
                                qk_chunk(i + 1, which, nb))
                    for st in range(4):
                        deferred_v.append(
                            lambda ii=i + 1, st=st: v_chunk(ii, st))
                # out-proj chunks available this strip: first half of the
                # previous strip's, deferred half of the one before (keeps
                # attention(3), which has no proj fillers, supplied with PE
                # work). Halves of one psum group stay adjacent.
                opc = []
                if i >= 1:
                    sts = (0, 1) if i < NSTRIP - 1 else (0, 1, 2, 3)
                    opc += [(i - 1, st, ec) for st in sts for ec in range(2)]
                if i >= 2:
                    opc += [(i - 2, st, ec) for st in (2, 3) for ec in range(2)]
                if i == NSTRIP - 1:
                    # Final strip: spread the filler units across all 72
                    # fill slots (1/pair + 1/head-end = 9 per head) instead
                    # of front-loading, so late heads keep the PE fed. Each
                    # outproj pair (nh0,nh1) stays adjacent (shared psum
                    # group). The ns0-1 out-proj halves pop at slot >= 41,
                    # after head 3's normalize is emitted — the PE queue is
                    # in-order, so popping earlier would head-of-line block
                    # attention matmuls on the OT dep.
                    sched = list(front)                     # slots 1-4
                    for ii, st, ec in opc:                  # slots 5-40
                        sched.append(lambda ii=ii, st=st, ec=ec:
                                     outproj_chunk(ii, st, ec, 0))
                        sched.append(lambda ii=ii, st=st, ec=ec:
                                     outproj_chunk(ii, st, ec, 1))
                        sched.append(lambda: None)
                    for st in range(4):                     # slots 41-72
                        for ec in range(2):
                            sched.append(lambda st=st, ec=ec:
                                         outproj3_half(st, ec, lo=True))
                            sched += [lambda: None] * 3
                    fillers = sched
                else:
                    if opc:
                        mixed = list(front)
                        front = []
                        fi = iter(fillers)
                        for ii, st, ec in opc:
                            mixed.append(lambda ii=ii, st=st, ec=ec:
                                         outproj_chunk(ii, st, ec, 0))
                            mixed.append(lambda ii=ii, st=st, ec=ec:
                                         outproj_chunk(ii, st, ec, 1))
                            got = 0
                            for _ in range(2):
                                try:
                                    mixed.append(next(fi))
                                    got += 1
                                except StopIteration:
                                    break
                            if i >= 2 and got == 0:
                                # spread bare units so strip-2's late heads
                                # keep the PE fed (same fix as strip 3)
                                mixed.append(lambda: None)
                        mixed.extend(fi)
                        fillers = mixed
                    fillers = front + fillers
                fit = iter(fillers)
                attention(i, fit)
                for f in fit:   # leftover fillers
                    f()

            # ---- final strip out-projection tail: only the ns2-3 half
            # remains; evacs split across Scalar+Vector and psum groups
            # alternate between the (now idle) psA2 banks and ps_mm.
            # Throwaway matmuls first: they run during the last head's
            # normalize chain (~3.4us of otherwise-idle PE) so the DVFS
            # clock stays ramped for the 16 real tail matmuls. ----
            wps = psum.tile([128, STRIP], FP32, name="warm", tag="psA2",
                            bufs=2)
            for k in range(16):
                nc.tensor.matmul(wps[0:8, :], lhsT=wo_sb[:, 0, 0:8],
                                 rhs=wo_sb[:, 0, 0:STRIP],
                                 start=(k == 0), stop=(k == 15))
            for st in range(4):
                for ec in range(2):
                    outproj3_half(st, ec, lo=False,
                                  ptag="psA2" if ec == 0 else "ps_mm",
                                  evac="s" if ec == 0 else "v")
    nc.compile()
    return nc


_CACHE = {}


def _tri_mask():
    # T[p, l, c] = 1.0 if c >= p else 0 (keep sq >= sk on diagonal corners)
    p = np.arange(128)[:, None, None]
    c = np.arange(128)[None, None, :]
    return np.broadcast_to(
        (c >= p), (128, 2, 128)).astype(np.float32).astype(ml_dtypes.bfloat16)


def kernel(x, W_qkv, b_qkv, W_o, b_o):
    x = np.ascontiguousarray(np.asarray(x, dtype=np.float32))
    W_qkv = np.asarray(W_qkv, dtype=np.float32)
    b_qkv = np.asarray(b_qkv, dtype=np.float32)
    W_o = np.asarray(W_o, dtype=np.float32)
    b_o = np.asarray(b_o, dtype=np.float32)

    if "nc" not in _CACHE:
        _CACHE["nc"] = build_bass()
    nc = _CACHE["nc"]

    in_maps = []
    for c in range(N_CORES):
        b, g = c // G, c % G
        n0 = g * NG
        bq = b_qkv[n0:n0 + NG]
        bk = b_qkv[D + n0:D + n0 + NG]
        bqk = np.concatenate(
            [bq.reshape(4, 128).T, bk.reshape(4, 128).T], axis=1)  # [128, 8]
        BF = ml_dtypes.bfloat16

        def _w(m):  # [D, NG] -> [128, DS, NG] contiguous bf16
            return m.reshape(DS, 128, -1).transpose(1, 0, 2).astype(BF)

        def _wh(m):  # [D, NG] -> [4, 128, DS, 128] (contiguous per-nb) bf16
            r = m.reshape(DS, 128, 4, 128)
            return r.transpose(2, 1, 0, 3).astype(BF)
        in_maps.append({
            "xt": x[b].T.astype(BF),
            "wq": _wh(W_qkv[:, n0:n0 + NG]),
            "wk": _wh(W_qkv[:, D + n0:D + n0 + NG]),
            "wv": _w(W_qkv[:, 2 * D + n0:2 * D + n0 + NG]),
            "bqk": np.ascontiguousarray(bqk),
            "bv": np.ascontiguousarray(
                b_qkv[2 * D + n0:2 * D + n0 + NG].reshape(1, NG)),
            "wo": W_o[n0:n0 + NG, :].reshape(4, 128, D).transpose(1, 0, 2)
                     .astype(BF),
            "tri": _tri_mask(),
        })

    _CACHE["in_maps"] = in_maps
    res = run_bass_kernel_spmd(nc, in_maps, list(range(N_CORES)))
    outs = res.results

    out = np.empty((B, S, D), dtype=np.float32)
    for b in range(B):
        out[b] = (outs[G * b]["out"].astype(np.float32)
                  + outs[G * b + 1]["out"].astype(np.float32))
        out[b][(NSTRIP - 1) * STRIP:] += (
            outs[G * b]["out2"].astype(np.float32)
            + outs[G * b + 1]["out2"].astype(np.float32))
    out += b_o[None, None, :]
    return out



# revision 43
# speedup vs baseline: 1.0204x; 1.0204x over previous
"""Causal multi-head attention on 8 Trainium2 NeuronCores.

Problem (hardcoded): x [4, 2048, 1024] fp32, W_qkv [1024, 3072], b_qkv [3072],
W_o [1024, 1024], b_o [1024]; 16 heads, head_dim 64.

Sharding: 8 cores = 4 batches x 2 head-groups (8 heads each). Each core
computes QKV projection for its (batch, head-group), causal attention for its
8 heads, and a partial out-projection [2048, 1024]. Host sums the two
head-group partials per batch and adds b_o.

Kernel strategy (per core, "transposed" domain):
  - x strip [512, 1024] -> PE-transpose -> xT [128, 8ds, 512]
  - QT/KTz = W^T x^T via matmul(lhsT=W_tile, rhs=xT); KTz zero-padded per
    head so the score matmul contracts K=128 (keeps PE at full clock).
  - V natural = matmul(lhsT=xT_tile, rhs=Wv), stored [128, blk, head, 65]
    with a ones column (denominator accumulates in psO row 64).
  - Scores per (head, strip): sk-blocks processed in PAIRS sharing one
    [128,1024] 2-bank psum tile; ONE Exp activation per off-diagonal pair.
    Diagonal blocks are causally trimmed (A/exp/AV restricted to sq>=128j)
    and masked via one strided 2-corner multiply with a [128,128] triangle.
  - Normalize: denominator copy + reciprocal_approx_fast + gpsimd
    partition_broadcast; OT = psO * recip (DVE).
  - out partial = matmul(lhsT=OT tile, rhs=W_o tiles) -> [s, e] -> DMA out.
  - Software pipelining: transposes/QKV-proj of strip i+1 and out-proj of
    strip i-1 are interleaved as PE fillers between attention pairs, so the
    PE fills gaps while Scalar (Exp) paces the attention inner loop.
Projection/out-proj matmuls run float32r; attention matmuls run bf16.
"""

import ml_dtypes
import numpy as np

import concourse.bass as bass
from concourse import bacc
import concourse.mybir as mybir
from concourse.bass_utils import run_bass_kernel_spmd
from concourse.tile import TileContext

B, S, D = 4, 2048, 1024
H, HD = 16, 64
G = 2                  # head groups (cores per batch)
HPG = H // G           # 8 heads per core
NG = HPG * HD          # 512 qkv feature columns per core
N_CORES = 8
STRIP = 512            # sq strip width
NSTRIP = S // STRIP    # 4
DS = D // 128          # 8 contraction subtiles for the projections
FP32 = mybir.dt.float32
R32 = mybir.dt.float32r
BF16 = mybir.dt.bfloat16
AF = mybir.ActivationFunctionType


def build_bass(dbg=False):
    nc = bacc.Bacc("TRN2")

    xt_d = nc.dram_tensor("xt", [D, S], BF16, kind="ExternalInput")
    wq_d = nc.dram_tensor("wq", [4, 128, DS, 128], BF16, kind="ExternalInput")
    wk_d = nc.dram_tensor("wk", [4, 128, DS, 128], BF16, kind="ExternalInput")
    wv_d = nc.dram_tensor("wv", [128, DS, NG], BF16, kind="ExternalInput")
    bqk_d = nc.dram_tensor("bqk", [128, 8], FP32, kind="ExternalInput")
    tri_d = nc.dram_tensor("tri", [128, 2, 128], BF16, kind="ExternalInput")
    bv_d = nc.dram_tensor("bv", [1, NG], FP32, kind="ExternalInput")
    wo_d = nc.dram_tensor("wo", [128, 4, D], BF16, kind="ExternalInput")
    out_d = nc.dram_tensor("out", [S, D], BF16, kind="ExternalOutput")
    # ns0-1 half-contraction partial of the LAST strip's out-projection,
    # computed mid-attention (hidden) and summed on the host; leaves only
    # the ns2-3 half + evac for the post-attention tail
    out2_d = nc.dram_tensor("out2", [STRIP, D], BF16, kind="ExternalOutput")

    with TileContext(nc) as tc:
        with (
            tc.tile_pool(name="const", bufs=1) as const,
            tc.tile_pool(name="persist", bufs=1) as persist,
            tc.tile_pool(name="work", bufs=2) as work,
            tc.tile_pool(name="psum", bufs=2, space="PSUM") as psum,
        ):
            xT = {}      # strip -> list of per-ds tiles

            def emit_xT_dmas(i):
                # x arrives pre-transposed from the host; one tile per ds so
                # the first projection matmul only waits for its own slice
                s0 = i * STRIP
                xT[i] = []
                for ds in range(DS):
                    t = work.tile([128, STRIP], BF16, name=f"xT{ds}",
                                  tag=f"xT{ds}", bufs=3)
                    nc.sync.dma_start(
                        t, xt_d[ds * 128:(ds + 1) * 128, s0:s0 + STRIP])
                    xT[i].append(t)

            # Startup DMA bandwidth (~200 GB/s aggregate) is the scarce
            # resource: issue transfers strictly in consumption order so the
            # critical ones aren't starved. Strip 1 prefetches in the
            # prologue too (xT bufs=3) so the strip-0 attention's projection
            # fillers aren't data-starved.
            emit_xT_dmas(0)
            bqk_sb = const.tile([128, 8], FP32, name="bqk_sb")
            nc.sync.dma_start(bqk_sb, bqk_d[:, :])
            bv_sb = const.tile([1, NG], FP32, name="bv_sb")
            nc.sync.dma_start(bv_sb, bv_d[:, :])
            emit_xT_dmas(1)
            tri2 = const.tile([128, 2, 128], BF16, name="tri2")
            nc.sync.dma_start(tri2, tri_d[:, :, :])
            wq_sb = [const.tile([128, DS, 128], BF16, name=f"wq_sb{q}")
                     for q in range(4)]
            wk_sb = [const.tile([128, DS, 128], BF16, name=f"wk_sb{q}")
                     for q in range(4)]
            wv_sb = const.tile([128, DS, NG], BF16, name="wv_sb")
            wo_sb = const.tile([128, 4, D], BF16, name="wo_sb")
            # weights stream on the Activation HWDGE queue in exact prologue
            # consumption order (q0,q1,k0,k1,q2,q3,k2,k3 then v)
            for q in (0, 1):
                nc.scalar.dma_start(wq_sb[q], wq_d[q])
            for q in (0, 1):
                nc.scalar.dma_start(wk_sb[q], wk_d[q])
            for q in (2, 3):
                nc.scalar.dma_start(wq_sb[q], wq_d[q])
            for q in (2, 3):
                nc.scalar.dma_start(wk_sb[q], wk_d[q])
            nc.scalar.dma_start(wv_sb, wv_d[:, :, :])
            # wo isn't consumed until the attention(1) out-proj fillers
            # (~60us): park it on the sync queue BEHIND the strip-1 x tiles
            # so it doesn't steal startup bandwidth from them
            nc.sync.dma_start(wo_sb, wo_d[:, :, :])
            # bias broadcast for the Vn evacuation add (one-time, on gpsimd)
            bvb = const.tile([128, NG], FP32, name="bvb")
            nc.gpsimd.partition_broadcast(bvb, bv_sb[0:1, :])

            # Persistent zero-padded K^T per head and V tiles (both bf16)
            KTz = persist.tile([128, HPG, S], BF16, name="KTz")
            # even heads occupy rows 0-63 (zero 64-127); odd heads vice versa
            for h in range(HPG):
                zrow = 64 if h % 2 == 0 else 0
                nc.gpsimd.memset(KTz[zrow:zrow + 64, h, :], 0.0)
            Vn = persist.tile([128, S // 128, HPG, HD + 1], BF16, name="Vn")
            nc.gpsimd.memset(Vn[:, :, :, HD], 1.0)

            QT = {}      # strip -> tile
            OT = {}      # strip -> tile

            def qk_chunk(i, which, nb):
                # 8 matmuls (full D contraction) + bias-add evacuation
                s0 = i * STRIP
                if which == 0 and nb == 0:
                    QT[i] = work.tile([128, 4, STRIP], BF16, name="QT",
                                      tag="QT", bufs=2)
                w_sb = (wq_sb if which == 0 else wk_sb)[nb]
                ps = psum.tile([128, STRIP], FP32, name="ps", tag="ps_mm",
                               bufs=2)
                for ds in range(DS):
                    nc.tensor.matmul(
                        ps, lhsT=w_sb[:, ds, :],
                        rhs=xT[i][ds],
                        start=(ds == 0), stop=(ds == DS - 1))
                bcol = bqk_sb[:, 4 * which + nb:4 * which + nb + 1]
                if which == 0:
                    nc.vector.tensor_scalar_add(QT[i][:, nb, :], ps, bcol)
                else:
                    nc.vector.tensor_scalar_add(
                        KTz[0:64, 2 * nb, s0:s0 + STRIP],
                        ps[0:64, :], bcol[0:64, :])
                    nc.vector.tensor_scalar_add(
                        KTz[64:128, 2 * nb + 1, s0:s0 + STRIP],
                        ps[64:128, :], bcol[64:128, :])

            def v_chunk(i, st):
                stg = i * 4 + st
                ps = psum.tile([128, STRIP], FP32, name="psv", tag="ps_mm",
                               bufs=2)
                for ds in range(DS):
                    nc.tensor.matmul(
                        ps,
                        lhsT=xT[i][ds][:, st * 128:(st + 1) * 128],
                        rhs=wv_sb[:, ds],
                        start=(ds == 0), stop=(ds == DS - 1))
                nc.vector.tensor_add(
                    Vn[:, stg, :, 0:HD],
                    ps.rearrange("p (h d) -> p h d", d=HD),
                    bvb.rearrange("p (h d) -> p h d", d=HD))

            ops_open = {}  # (i, st, ec) -> psum tile across half-chunks

            def outproj_chunk(i, st, ec, nh, evac="v", ptag="ps_mm"):
                # half-chunk: ns 0-1 (nh=0) opens the psum group, ns 2-3
                # (nh=1) closes it, evacuates, and DMAs its half out.
                # One [128,512] ob tile per (st,ec) half (bufs=4) so a
                # block's evacuation never WAR-waits on the previous
                # block's 512KB out-DMA.
                s0 = i * STRIP
                if nh == 0:
                    ps = psum.tile([128, STRIP], FP32, name="pso",
                                   tag=ptag, bufs=2)
                    ops_open[(i, st, ec)] = ps
                else:
                    ps = ops_open.pop((i, st, ec))
                for ns in (2 * nh, 2 * nh + 1):
                    nc.tensor.matmul(
                        ps,
                        lhsT=OT[i][:, ns, st * 128:(st + 1) * 128],
                        rhs=wo_sb[:, ns, ec * 512:(ec + 1) * 512],
                        start=(ns == 0), stop=(ns == 3))
                if nh == 1:
                    obt = work.tile([128, STRIP], BF16, name="ob", tag="ob",
                                    bufs=4)
                    # DMA trigger on gpsimd keeps the (busy) Scalar queue
                    # free of DIRECT2D descriptor writes
                    if evac == "s":
                        nc.scalar.copy(obt, ps)
                        trig = nc.scalar
                    else:
                        nc.vector.tensor_copy(obt, ps)
                        trig = nc.gpsimd
                    trig.dma_start(
                        out_d[s0 + st * 128:s0 + (st + 1) * 128,
                              ec * 512:(ec + 1) * 512], obt)

            def attention(i, fillers):
                def fill():
                    try:
                        next(fillers)()
                    except StopIteration:
                        pass

                npair = 2 * i + 2  # 2i off-diagonal pairs + 2 diagonal pairs
                OT[i] = work.tile([128, 4, STRIP], BF16, name="OT", tag="OT",
                                  bufs=3)
                for h in range(HPG):
                    prow = (h % 2) * 64
                    nsub = h // 2
                    psO = psum.tile([128, STRIP], FP32, name="psO", tag="psO",
                                    bufs=2)

                    def emit_avs(pend):
                        # AVs for an already-exp'd pair (one-pair lookahead:
                        # by now the exp/corner-mul are long done, so these
                        # issue without stalling the PE)
                        p, expP = pend
                        for l in range(2):
                            if p < 2 * i:
                                b, c0, j = 2 * p + l, 512 * l, 0
                            else:
                                j = 2 * (p - 2 * i) + l
                                b, c0 = 4 * i + j, 512 * l + 128 * j
                            nc.tensor.matmul(
                                psO[0:HD + 1, 128 * j:STRIP],
                                lhsT=Vn[:, b, h, :],
                                rhs=expP[:, c0:512 * (l + 1)],
                                start=(i == 0 and p == 0 and l == 0)
                                or (i > 0 and p == 0 and l == 0),
                                stop=(p == npair - 1 and l == 1),
                                skip_group_check=True)

                    pending = None
                    for p in range(npair):
                        psA = psum.tile([128, 1024], FP32, name="psA",
                                        tag="psA2", bufs=2)
                        expP = work.tile([128, 1024], BF16, name="expP",
                                         tag="expP", bufs=6)
                        if p < 2 * i:        # off-diagonal pair, full width
                            for l in range(2):
                                b = 2 * p + l
                                nc.tensor.matmul(
                                    psA[:, 512 * l:512 * (l + 1)],
                                    lhsT=KTz[:, h, b * 128:(b + 1) * 128],
                                    rhs=QT[i][:, nsub, :],
                                    start=True, stop=True)
                            nc.scalar.activation(expP, psA, AF.Exp,
                                                 scale=0.125)
                        else:                # diagonal pair, causally trimmed
                            pd = p - 2 * i
                            for l in range(2):
                                j = 2 * pd + l
                                b = 4 * i + j
                                c0 = 512 * l + 128 * j
                                nc.tensor.matmul(
                                    psA[:, c0:512 * (l + 1)],
                                    lhsT=KTz[:, h, b * 128:(b + 1) * 128],
                                    rhs=QT[i][:, nsub, 128 * j:STRIP],
                                    start=True, stop=True,
                                    skip_group_check=True)
                            # per-slot exps: skip the stale psum gap
                            # [512, 640+256pd) between the two trimmed slots
                            nc.scalar.activation(
                                expP[:, 256 * pd:512],
                                psA[:, 256 * pd:512],
                                AF.Exp, scale=0.125)
                            nc.scalar.activation(
                                expP[:, 640 + 256 * pd:1024],
                                psA[:, 640 + 256 * pd:1024],
                                AF.Exp, scale=0.125)
                            # strided 2-corner causal mask multiply
                            cbase = expP[:, 256 * pd:256 * pd + 768]
                            cap = bass.AP(
                                tensor=cbase.tensor, offset=cbase.offset,
                                ap=[list(cbase.ap[0])] + [[640, 2], [1, 128]])
                            nc.vector.tensor_mul(cap, cap, tri2)
                        if pending is not None:
                            emit_avs(pending)
                        pending = (p, expP)
                        fill()
                    emit_avs(pending)
                    # normalize: recip of denominator row, broadcast, mult.
                    # The very last head's den copy rides Scalar (idle after
                    # the final exp) to shorten the exposed tail chain.
                    den = work.tile([1, STRIP], FP32, name="den", tag="den",
                                    bufs=1)
                    if i == NSTRIP - 1 and h == HPG - 1:
                        nc.scalar.copy(den, psO[HD:HD + 1, :])
                    else:
                        nc.vector.tensor_copy(den, psO[HD:HD + 1, :])
                    recip = work.tile([1, STRIP], FP32, name="recip",
                                      tag="recip", bufs=1)
                    nc.vector.reciprocal_approx_fast(recip, den)
                    pbt = work.tile([64, STRIP], FP32, name="pbt", tag="pbt",
                                    bufs=2)
                    nc.gpsimd.partition_broadcast(pbt, recip[0:1, :])
                    nc.vector.tensor_mul(OT[i][prow:prow + 64, nsub, :],
                                         psO[0:HD, :], pbt)
                    fill()

            # ---- prologue: strip 0 projections, ordered to match DMA
            # arrival; all QK first so attention(0)'s score/exp stream
            # starts as early as possible, V last (first AV consumes Vn
            # only a pair later) ----
            for nb in range(2):
                qk_chunk(0, 0, nb)
            for nb in range(2):
                qk_chunk(0, 1, nb)
            for nb in range(2, 4):
                qk_chunk(0, 0, nb)
            for nb in range(2, 4):
                qk_chunk(0, 1, nb)
            for st in range(4):
                v_chunk(0, st)

            def outproj3_half(st, ec, lo, ptag="ps_mm", evac="v"):
                # half-contraction out-proj for the final strip: lo=True does
                # ns 0-1 -> out2 partial (hidden mid-attention), lo=False does
                # ns 2-3 -> out (the only post-attention PE work)
                i = NSTRIP - 1
                nss = (0, 1) if lo else (2, 3)
                ps = psum.tile([128, STRIP], FP32, name="ps3", tag=ptag,
                               bufs=2)
                for ns in nss:
                    nc.tensor.matmul(
                        ps,
                        lhsT=OT[i][:, ns, st * 128:(st + 1) * 128],
                        rhs=wo_sb[:, ns, ec * 512:(ec + 1) * 512],
                        start=(ns == nss[0]), stop=(ns == nss[1]))
                obt = work.tile([128, STRIP], BF16, name="ob", tag="ob",
                                bufs=4)
                if evac == "s":
                    nc.scalar.copy(obt, ps)
                    trig = nc.scalar
                else:
                    nc.vector.tensor_copy(obt, ps)
                    trig = nc.gpsimd
                if lo:
                    dst = out2_d[st * 128:(st + 1) * 128,
                                 ec * 512:(ec + 1) * 512]
                else:
                    s0 = i * STRIP
                    dst = out_d[s0 + st * 128:s0 + (st + 1) * 128,
                                ec * 512:(ec + 1) * 512]
                trig.dma_start(dst, obt)

            # ---- main loop: attention(i) with interleaved fillers ----
            # v chunks of strip i are deferred INTO attention(i): the first
            # diagonal AV that reads Vn block 4i+j is emitted >= 2 fill
            # slots after the head starts (one-pair lookahead), so v(i)
            # popped in the first 4 slots always lands in time. This keeps
            # the late, filler-starved strips supplied with PE work.
            deferred_v = []
            for i in range(NSTRIP):
                front = list(deferred_v)   # MUST be the first pops (Vn deps)
                fillers = []
                deferred_v = []
                if i + 2 < NSTRIP:
                    emit_xT_dmas(i + 2)  # strips 0/1 prefetched in prologue
                if i + 1 < NSTRIP:
                    for which in range(2):
                        for nb in range(4):
                            fillers.append(
                                lambda which=which, nb=nb:
                                qk_chunk(i + 1, which, nb))
                    for st in range(4):
                        deferred_v.append(
                            lambda ii=i + 1, st=st: v_chunk(ii, st))
                # out-proj chunks available this strip: first half of the
                # previous strip's, deferred half of the one before (keeps
                # attention(3), which has no proj fillers, supplied with PE
                # work). Halves of one psum group stay adjacent.
                opc = []
                if i >= 1:
                    sts = (0, 1) if i < NSTRIP - 1 else (0, 1, 2, 3)
                    opc += [(i - 1, st, ec) for st in sts for ec in range(2)]
                if i >= 2:
                    opc += [(i - 2, st, ec) for st in (2, 3) for ec in range(2)]
                if i == NSTRIP - 1:
                    # Final strip: spread the filler units across all 72
                    # fill slots (1/pair + 1/head-end = 9 per head) instead
                    # of front-loading, so late heads keep the PE fed. Each
                    # outproj pair (nh0,nh1) stays adjacent (shared psum
                    # group). The ns0-1 out-proj halves pop at slot >= 41,
                    # after head 3's normalize is emitted — the PE queue is
                    # in-order, so popping earlier would head-of-line block
                    # attention matmuls on the OT dep.
                    sched = list(front)                     # slots 1-4
                    for ii, st, ec in opc:                  # slots 5-40
                        sched.append(lambda ii=ii, st=st, ec=ec:
                                     outproj_chunk(ii, st, ec, 0))
                        sched.append(lambda ii=ii, st=st, ec=ec:
                                     outproj_chunk(ii, st, ec, 1))
                        sched.append(lambda: None)
                    for st in range(4):                     # slots 41-72
                        for ec in range(2):
                            sched.append(lambda st=st, ec=ec:
                                         outproj3_half(st, ec, lo=True))
                            sched += [lambda: None] * 3
                    fillers = sched
                else:
                    if opc:
                        mixed = list(front)
                        front = []
                        fi = iter(fillers)
                        for ii, st, ec in opc:
                            mixed.append(lambda ii=ii, st=st, ec=ec:
                                         outproj_chunk(ii, st, ec, 0))
                            mixed.append(lambda ii=ii, st=st, ec=ec:
                                         outproj_chunk(ii, st, ec, 1))
                            got = 0
                            for _ in range(2):
                                try:
                                    mixed.append(next(fi))
                                    got += 1
                                except StopIteration:
                                    break
                            if i >= 2 and got == 0:
                                # spread bare units so strip-2's late heads
                                # keep the PE fed (same fix as strip 3)
                                mixed.append(lambda: None)
                        mixed.extend(fi)
                        fillers = mixed
                    fillers = front + fillers
                fit = iter(fillers)
                attention(i, fit)
                for f in fit:   # leftover fillers
                    f()

            # ---- final strip out-projection tail: only the ns2-3 half
            # remains; evacs split across Scalar+Vector and psum groups
            # alternate between the (now idle) psA2 banks and ps_mm.
            # Throwaway matmuls first: they run during the last head's
            # normalize chain (~3.4us of otherwise-idle PE) so the DVFS
            # clock stays ramped for the 16 real tail matmuls. ----
            wps = psum.tile([128, STRIP], FP32, name="warm", tag="psA2",
                            bufs=2)
            for k in range(16):
                nc.tensor.matmul(wps[0:8, :], lhsT=wo_sb[:, 0, 0:8],
                                 rhs=wo_sb[:, 0, 0:STRIP],
                                 start=(k == 0), stop=(k == 15))
            for st in range(4):
                for ec in range(2):
                    outproj3_half(st, ec, lo=False,
                                  ptag="psA2" if ec == 0 else "ps_mm",
                                  evac="s" if ec == 0 else "v")
    nc.compile()
    return nc


_CACHE = {}


def _tri_mask():
    # T[p, l, c] = 1.0 if c >= p else 0 (keep sq >= sk on diagonal corners)
    p = np.arange(128)[:, None, None]
    c = np.arange(128)[None, None, :]
    return np.broadcast_to(
        (c >= p), (128, 2, 128)).astype(np.float32).astype(ml_dtypes.bfloat16)


def kernel(x, W_qkv, b_qkv, W_o, b_o):
    x = np.ascontiguousarray(np.asarray(x, dtype=np.float32))
    W_qkv = np.asarray(W_qkv, dtype=np.float32)
    b_qkv = np.asarray(b_qkv, dtype=np.float32)
    W_o = np.asarray(W_o, dtype=np.float32)
    b_o = np.asarray(b_o, dtype=np.float32)

    if "nc" not in _CACHE:
        _CACHE["nc"] = build_bass()
    nc = _CACHE["nc"]

    in_maps = []
    for c in range(N_CORES):
        b, g = c // G, c % G
        n0 = g * NG
        bq = b_qkv[n0:n0 + NG]
        bk = b_qkv[D + n0:D + n0 + NG]
        bqk = np.concatenate(
            [bq.reshape(4, 128).T, bk.reshape(4, 128).T], axis=1)  # [128, 8]
        BF = ml_dtypes.bfloat16

        def _w(m):  # [D, NG] -> [128, DS, NG] contiguous bf16
            return m.reshape(DS, 128, -1).transpose(1, 0, 2).astype(BF)

        def _wh(m):  # [D, NG] -> [4, 128, DS, 128] (contiguous per-nb) bf16
            r = m.reshape(DS, 128, 4, 128)
            return r.transpose(2, 1, 0, 3).astype(BF)
        in_maps.append({
            "xt": x[b].T.astype(BF),
            "wq": _wh(W_qkv[:, n0:n0 + NG]),
            "wk": _wh(W_qkv[:, D + n0:D + n0 + NG]),
            "wv": _w(W_qkv[:, 2 * D + n0:2 * D + n0 + NG]),
            "bqk": np.ascontiguousarray(bqk),
            "bv": np.ascontiguousarray(
                b_qkv[2 * D + n0:2 * D + n0 + NG].reshape(1, NG)),
            "wo": W_o[n0:n0 + NG, :].reshape(4, 128, D).transpose(1, 0, 2)
                     .astype(BF),
            "tri": _tri_mask(),
        })

    _CACHE["in_maps"] = in_maps
    res = run_bass_kernel_spmd(nc, in_maps, list(range(N_CORES)))
    outs = res.results

    out = np.empty((B, S, D), dtype=np.float32)
    for b in range(B):
        out[b] = (outs[G * b]["out"].astype(np.float32)
                  + outs[G * b + 1]["out"].astype(np.float32))
        out[b][(NSTRIP - 1) * STRIP:] += (
            outs[G * b]["out2"].astype(np.float32)
            + outs[G * b + 1]["out2"].astype(np.float32))
    out += b_o[None, None, :]
    return out



# revision 45
# speedup vs baseline: 1.0263x; 1.0058x over previous
"""Causal multi-head attention on 8 Trainium2 NeuronCores.

Problem (hardcoded): x [4, 2048, 1024] fp32, W_qkv [1024, 3072], b_qkv [3072],
W_o [1024, 1024], b_o [1024]; 16 heads, head_dim 64.

Sharding: 8 cores = 4 batches x 2 head-groups (8 heads each). Each core
computes QKV projection for its (batch, head-group), causal attention for its
8 heads, and a partial out-projection [2048, 1024]. Host sums the two
head-group partials per batch (plus a separate ns0-1 partial of the final
strip, see below) and adds b_o.

Kernel strategy (per core, "transposed" domain):
  - x arrives host-transposed+bf16: xT tiles [128, 512] per ds-chunk.
  - QT/KTz = W^T x^T via matmul(lhsT=W_tile, rhs=xT); KTz zero-padded per
    head so the score matmul contracts K=128 (keeps PE at full clock).
  - V natural = matmul(lhsT=xT_tile, rhs=Wv), stored [128, blk, head, 65]
    with a ones column (denominator accumulates in psO row 64).
  - Scores per (head, strip): sk-blocks processed in PAIRS sharing one
    [128,1024] 2-bank psum tile; per-slot Exp activations skip the stale
    psum gap on diagonal pairs. Diagonal blocks are causally trimmed
    (A/exp/AV restricted to sq>=128j) and masked via one strided 2-corner
    multiply with a [128,128] triangle.
  - Normalize: denominator copy + reciprocal_approx_fast + gpsimd
    partition_broadcast; OT = psO * recip (DVE).
  - out partial = matmul(lhsT=OT tile, rhs=W_o tiles) -> [s, e] -> DMA out.
  - Software pipelining: QKV-proj of strip i+1 and out-proj of strips
    i-1/i-2 are interleaved as PE fillers between attention pairs (spread
    across all fill slots so late heads keep the PE fed). The final
    strip's out-proj is split at the contraction: the ns0-1 half runs
    mid-attention into a separate `out2` DRAM partial (host sums 3
    partials for those rows), so only the ns2-3 half + evac trails the
    last head; throwaway matmuls keep the PE's DVFS clock ramped through
    the last normalize chain.
  - Startup: DMA bandwidth (~200 GB/s aggregate) is the scarce resource;
    transfers issue in exact consumption order (x strip 0, biases,
    x strip 1, tri, wo on the sync ring | per-nb contiguous wq/wk then wv
    on the act ring). All-QK-first prologue starts the score/exp stream
    as early as possible.
Everything runs bf16 (inputs/weights host-converted; fp32 psum accumulate,
fp32 softmax denominator/reciprocal); out partials are bf16, summed fp32
on the host. Measured rel err 2.8e-3 vs fp32 reference.
"""

import ml_dtypes
import numpy as np

import concourse.bass as bass
from concourse import bacc
import concourse.mybir as mybir
from concourse.bass_utils import run_bass_kernel_spmd
from concourse.tile import TileContext

B, S, D = 4, 2048, 1024
H, HD = 16, 64
G = 2                  # head groups (cores per batch)
HPG = H // G           # 8 heads per core
NG = HPG * HD          # 512 qkv feature columns per core
N_CORES = 8
STRIP = 512            # sq strip width
NSTRIP = S // STRIP    # 4
DS = D // 128          # 8 contraction subtiles for the projections
FP32 = mybir.dt.float32
BF16 = mybir.dt.bfloat16
AF = mybir.ActivationFunctionType


def build_bass(dbg=False):
    nc = bacc.Bacc("TRN2")

    xt_d = nc.dram_tensor("xt", [D, S], BF16, kind="ExternalInput")
    wq_d = nc.dram_tensor("wq", [4, 128, DS, 128], BF16, kind="ExternalInput")
    wk_d = nc.dram_tensor("wk", [4, 128, DS, 128], BF16, kind="ExternalInput")
    wv_d = nc.dram_tensor("wv", [128, DS, NG], BF16, kind="ExternalInput")
    bqk_d = nc.dram_tensor("bqk", [128, 8], FP32, kind="ExternalInput")
    tri_d = nc.dram_tensor("tri", [128, 2, 128], BF16, kind="ExternalInput")
    bv_d = nc.dram_tensor("bv", [1, NG], FP32, kind="ExternalInput")
    wo_d = nc.dram_tensor("wo", [128, 4, D], BF16, kind="ExternalInput")
    out_d = nc.dram_tensor("out", [S, D], BF16, kind="ExternalOutput")
    # ns0-1 half-contraction partial of the LAST strip's out-projection,
    # computed mid-attention (hidden) and summed on the host; leaves only
    # the ns2-3 half + evac for the post-attention tail
    out2_d = nc.dram_tensor("out2", [STRIP, D], BF16, kind="ExternalOutput")

    with TileContext(nc) as tc:
        with (
            tc.tile_pool(name="const", bufs=1) as const,
            tc.tile_pool(name="persist", bufs=1) as persist,
            tc.tile_pool(name="work", bufs=2) as work,
            tc.tile_pool(name="psum", bufs=2, space="PSUM") as psum,
        ):
            xT = {}      # strip -> list of per-ds tiles

            def emit_xT_dmas(i):
                # x arrives pre-transposed from the host; one tile per ds so
                # the first projection matmul only waits for its own slice
                s0 = i * STRIP
                xT[i] = []
                for ds in range(DS):
                    t = work.tile([128, STRIP], BF16, name=f"xT{ds}",
                                  tag=f"xT{ds}", bufs=3)
                    nc.sync.dma_start(
                        t, xt_d[ds * 128:(ds + 1) * 128, s0:s0 + STRIP])
                    xT[i].append(t)

            # Startup DMA bandwidth (~200 GB/s aggregate) is the scarce
            # resource: issue transfers strictly in consumption order so the
            # critical ones aren't starved. Strip 1 prefetches in the
            # prologue too (xT bufs=3) so the strip-0 attention's projection
            # fillers aren't data-starved.
            emit_xT_dmas(0)
            bqk_sb = const.tile([128, 8], FP32, name="bqk_sb")
            nc.sync.dma_start(bqk_sb, bqk_d[:, :])
            bv_sb = const.tile([1, NG], FP32, name="bv_sb")
            nc.sync.dma_start(bv_sb, bv_d[:, :])
            emit_xT_dmas(1)
            tri2 = const.tile([128, 2, 128], BF16, name="tri2")
            nc.sync.dma_start(tri2, tri_d[:, :, :])
            wq_sb = [const.tile([128, DS, 128], BF16, name=f"wq_sb{q}")
                     for q in range(4)]
            wk_sb = [const.tile([128, DS, 128], BF16, name=f"wk_sb{q}")
                     for q in range(4)]
            wv_sb = const.tile([128, DS, NG], BF16, name="wv_sb")
            wo_sb = const.tile([128, 4, D], BF16, name="wo_sb")
            # weights stream on the Activation HWDGE queue in exact prologue
            # consumption order (q0,q1,k0,k1,q2,q3,k2,k3 then v)
            for q in (0, 1):
                nc.scalar.dma_start(wq_sb[q], wq_d[q])
            for q in (0, 1):
                nc.scalar.dma_start(wk_sb[q], wk_d[q])
            for q in (2, 3):
                nc.scalar.dma_start(wq_sb[q], wq_d[q])
            for q in (2, 3):
                nc.scalar.dma_start(wk_sb[q], wk_d[q])
            nc.scalar.dma_start(wv_sb, wv_d[:, :, :])
            # wo isn't consumed until the attention(1) out-proj fillers
            # (~60us): park it on the sync queue BEHIND the strip-1 x tiles
            # so it doesn't steal startup bandwidth from them
            nc.sync.dma_start(wo_sb, wo_d[:, :, :])
            # bias broadcast for the Vn evacuation add (one-time, on gpsimd)
            bvb = const.tile([128, NG], FP32, name="bvb")
            nc.gpsimd.partition_broadcast(bvb, bv_sb[0:1, :])

            # Persistent zero-padded K^T per head and V tiles (both bf16)
            KTz = persist.tile([128, HPG, S], BF16, name="KTz")
            # even heads occupy rows 0-63 (zero 64-127); odd heads vice versa
            for h in range(HPG):
                zrow = 64 if h % 2 == 0 else 0
                nc.gpsimd.memset(KTz[zrow:zrow + 64, h, :], 0.0)
            Vn = persist.tile([128, S // 128, HPG, HD + 1], BF16, name="Vn")
            nc.gpsimd.memset(Vn[:, :, :, HD], 1.0)

            QT = {}      # strip -> tile
            OT = {}      # strip -> tile

            def qk_chunk(i, which, nb):
                # 8 matmuls (full D contraction) + bias-add evacuation
                s0 = i * STRIP
                if which == 0 and nb == 0:
                    QT[i] = work.tile([128, 4, STRIP], BF16, name="QT",
                                      tag="QT", bufs=2)
                w_sb = (wq_sb if which == 0 else wk_sb)[nb]
                ps = psum.tile([128, STRIP], FP32, name="ps", tag="ps_mm",
                               bufs=2)
                for ds in range(DS):
                    nc.tensor.matmul(
                        ps, lhsT=w_sb[:, ds, :],
                        rhs=xT[i][ds],
                        start=(ds == 0), stop=(ds == DS - 1))
                bcol = bqk_sb[:, 4 * which + nb:4 * which + nb + 1]
                if which == 0:
                    nc.vector.tensor_scalar_add(QT[i][:, nb, :], ps, bcol)
                else:
                    nc.vector.tensor_scalar_add(
                        KTz[0:64, 2 * nb, s0:s0 + STRIP],
                        ps[0:64, :], bcol[0:64, :])
                    nc.vector.tensor_scalar_add(
                        KTz[64:128, 2 * nb + 1, s0:s0 + STRIP],
                        ps[64:128, :], bcol[64:128, :])

            def v_chunk(i, st):
                stg = i * 4 + st
                ps = psum.tile([128, STRIP], FP32, name="psv", tag="ps_mm",
                               bufs=2)
                for ds in range(DS):
                    nc.tensor.matmul(
                        ps,
                        lhsT=xT[i][ds][:, st * 128:(st + 1) * 128],
                        rhs=wv_sb[:, ds],
                        start=(ds == 0), stop=(ds == DS - 1))
                nc.vector.tensor_add(
                    Vn[:, stg, :, 0:HD],
                    ps.rearrange("p (h d) -> p h d", d=HD),
                    bvb.rearrange("p (h d) -> p h d", d=HD))

            ops_open = {}  # (i, st, ec) -> psum tile across half-chunks

            def outproj_chunk(i, st, ec, nh, evac="v", ptag="ps_mm"):
                # half-chunk: ns 0-1 (nh=0) opens the psum group, ns 2-3
                # (nh=1) closes it, evacuates, and DMAs its half out.
                # One [128,512] ob tile per (st,ec) half (bufs=4) so a
                # block's evacuation never WAR-waits on the previous
                # block's 512KB out-DMA.
                s0 = i * STRIP
                if nh == 0:
                    ps = psum.tile([128, STRIP], FP32, name="pso",
                                   tag=ptag, bufs=2)
                    ops_open[(i, st, ec)] = ps
                else:
                    ps = ops_open.pop((i, st, ec))
                for ns in (2 * nh, 2 * nh + 1):
                    nc.tensor.matmul(
                        ps,
                        lhsT=OT[i][:, ns, st * 128:(st + 1) * 128],
                        rhs=wo_sb[:, ns, ec * 512:(ec + 1) * 512],
                        start=(ns == 0), stop=(ns == 3))
                if nh == 1:
                    obt = work.tile([128, STRIP], BF16, name="ob", tag="ob",
                                    bufs=4)
                    # DMA trigger on gpsimd keeps the (busy) Scalar queue
                    # free of DIRECT2D descriptor writes
                    if evac == "s":
                        nc.scalar.copy(obt, ps)
                        trig = nc.scalar
                    else:
                        nc.vector.tensor_copy(obt, ps)
                        trig = nc.gpsimd
                    trig.dma_start(
                        out_d[s0 + st * 128:s0 + (st + 1) * 128,
                              ec * 512:(ec + 1) * 512], obt)

            def attention(i, fillers):
                def fill():
                    try:
                        next(fillers)()
                    except StopIteration:
                        pass

                npair = 2 * i + 2  # 2i off-diagonal pairs + 2 diagonal pairs
                OT[i] = work.tile([128, 4, STRIP], BF16, name="OT", tag="OT",
                                  bufs=3)
                for h in range(HPG):
                    prow = (h % 2) * 64
                    nsub = h // 2
                    psO = psum.tile([128, STRIP], FP32, name="psO", tag="psO",
                                    bufs=2)

                    def emit_avs(pend):
                        # AVs for an already-exp'd pair (one-pair lookahead:
                        # by now the exp/corner-mul are long done, so these
                        # issue without stalling the PE)
                        p, expP = pend
                        for l in range(2):
                            if p < 2 * i:
                                b, c0, j = 2 * p + l, 512 * l, 0
                            else:
                                j = 2 * (p - 2 * i) + l
                                b, c0 = 4 * i + j, 512 * l + 128 * j
                            nc.tensor.matmul(
                                psO[0:HD + 1, 128 * j:STRIP],
                                lhsT=Vn[:, b, h, :],
                                rhs=expP[:, c0:512 * (l + 1)],
                                start=(i == 0 and p == 0 and l == 0)
                                or (i > 0 and p == 0 and l == 0),
                                stop=(p == npair - 1 and l == 1),
                                skip_group_check=True)

                    pending = None
                    for p in range(npair):
                        psA = psum.tile([128, 1024], FP32, name="psA",
                                        tag="psA2", bufs=2)
                        expP = work.tile([128, 1024], BF16, name="expP",
                                         tag="expP", bufs=6)
                        if p < 2 * i:        # off-diagonal pair, full width
                            for l in range(2):
                                b = 2 * p + l
                                nc.tensor.matmul(
                                    psA[:, 512 * l:512 * (l + 1)],
                                    lhsT=KTz[:, h, b * 128:(b + 1) * 128],
                                    rhs=QT[i][:, nsub, :],
                                    start=True, stop=True)
                            nc.scalar.activation(expP, psA, AF.Exp,
                                                 scale=0.125)
                        else:                # diagonal pair, causally trimmed
                            pd = p - 2 * i
                            for l in range(2):
                                j = 2 * pd + l
                                b = 4 * i + j
                                c0 = 512 * l + 128 * j
                                nc.tensor.matmul(
                                    psA[:, c0:512 * (l + 1)],
                                    lhsT=KTz[:, h, b * 128:(b + 1) * 128],
                                    rhs=QT[i][:, nsub, 128 * j:STRIP],
                                    start=True, stop=True,
                                    skip_group_check=True)
                            # per-slot exps: skip the stale psum gap
                            # [512, 640+256pd) between the two trimmed slots
                            nc.scalar.activation(
                                expP[:, 256 * pd:512],
                                psA[:, 256 * pd:512],
                                AF.Exp, scale=0.125)
                            nc.scalar.activation(
                                expP[:, 640 + 256 * pd:1024],
                                psA[:, 640 + 256 * pd:1024],
                                AF.Exp, scale=0.125)
                            # strided 2-corner causal mask multiply
                            cbase = expP[:, 256 * pd:256 * pd + 768]
                            cap = bass.AP(
                                tensor=cbase.tensor, offset=cbase.offset,
                                ap=[list(cbase.ap[0])] + [[640, 2], [1, 128]])
                            nc.vector.tensor_mul(cap, cap, tri2)
                        if pending is not None:
                            emit_avs(pending)
                        pending = (p, expP)
                        fill()
                    emit_avs(pending)
                    # normalize: recip of denominator row, broadcast, mult.
                    # The very last head's den copy rides Scalar (idle after
                    # the final exp) to shorten the exposed tail chain.
                    den = work.tile([1, STRIP], FP32, name="den", tag="den",
                                    bufs=1)
                    if i == NSTRIP - 1 and h == HPG - 1:
                        nc.scalar.copy(den, psO[HD:HD + 1, :])
                    else:
                        nc.vector.tensor_copy(den, psO[HD:HD + 1, :])
                    recip = work.tile([1, STRIP], FP32, name="recip",
                                      tag="recip", bufs=1)
                    nc.vector.reciprocal_approx_fast(recip, den)
                    pbt = work.tile([64, STRIP], FP32, name="pbt", tag="pbt",
                                    bufs=2)
                    nc.gpsimd.partition_broadcast(pbt, recip[0:1, :])
                    nc.vector.tensor_mul(OT[i][prow:prow + 64, nsub, :],
                                         psO[0:HD, :], pbt)
                    fill()

            # ---- prologue: strip 0 projections, ordered to match DMA
            # arrival; all QK first so attention(0)'s score/exp stream
            # starts as early as possible, V last (first AV consumes Vn
            # only a pair later) ----
            for nb in range(2):
                qk_chunk(0, 0, nb)
            for nb in range(2):
                qk_chunk(0, 1, nb)
            for nb in range(2, 4):
                qk_chunk(0, 0, nb)
            for nb in range(2, 4):
                qk_chunk(0, 1, nb)
            for st in range(4):
                v_chunk(0, st)

            def outproj3_half(st, ec, lo, ptag="ps_mm", evac="v"):
                # half-contraction out-proj for the final strip: lo=True does
                # ns 0-1 -> out2 partial (hidden mid-attention), lo=False does
                # ns 2-3 -> out (the only post-attention PE work)
                i = NSTRIP - 1
                nss = (0, 1) if lo else (2, 3)
                ps = psum.tile([128, STRIP], FP32, name="ps3", tag=ptag,
                               bufs=2)
                for ns in nss:
                    nc.tensor.matmul(
                        ps,
                        lhsT=OT[i][:, ns, st * 128:(st + 1) * 128],
                        rhs=wo_sb[:, ns, ec * 512:(ec + 1) * 512],
                        start=(ns == nss[0]), stop=(ns == nss[1]))
                obt = work.tile([128, STRIP], BF16, name="ob", tag="ob",
                                bufs=4)
                if evac == "s":
                    nc.scalar.copy(obt, ps)
                    trig = nc.scalar
                else:
                    nc.vector.tensor_copy(obt, ps)
                    trig = nc.gpsimd
                if lo:
                    dst = out2_d[st * 128:(st + 1) * 128,
                                 ec * 512:(ec + 1) * 512]
                else:
                    s0 = i * STRIP
                    dst = out_d[s0 + st * 128:s0 + (st + 1) * 128,
                                ec * 512:(ec + 1) * 512]
                trig.dma_start(dst, obt)

            # ---- main loop: attention(i) with interleaved fillers ----
            # v chunks of strip i are deferred INTO attention(i): the first
            # diagonal AV that reads Vn block 4i+j is emitted >= 2 fill
            # slots after the head starts (one-pair lookahead), so v(i)
            # popped in the first 4 slots always lands in time. This keeps
            # the late, filler-starved strips supplied with PE work.
            deferred_v = []
            for i in range(NSTRIP):
                front = list(deferred_v)   # MUST be the first pops (Vn deps)
                fillers = []
                deferred_v = []
                if i + 2 < NSTRIP:
                    emit_xT_dmas(i + 2)  # strips 0/1 prefetched in prologue
                if i + 1 < NSTRIP:
                    for which in range(2):
                        for nb in range(4):
                            fillers.append(
                                lambda which=which, nb=nb:
                                qk_chunk(i + 1, which, nb))
                    for st in range(4):
                        deferred_v.append(
                            lambda ii=i + 1, st=st: v_chunk(ii, st))
                # out-proj chunks available this strip: first half of the
                # previous strip's, deferred half of the one before (keeps
                # attention(3), which has no proj fillers, supplied with PE
                # work). Halves of one psum group stay adjacent.
                opc = []
                if i >= 1:
                    sts = (0, 1) if i < NSTRIP - 1 else (0, 1, 2, 3)
                    opc += [(i - 1, st, ec) for st in sts for ec in range(2)]
                if i >= 2:
                    opc += [(i - 2, st, ec) for st in (2, 3) for ec in range(2)]
                if i == NSTRIP - 1:
                    # Final strip: spread the filler units across all 72
                    # fill slots (1/pair + 1/head-end = 9 per head) instead
                    # of front-loading, so late heads keep the PE fed. Each
                    # outproj pair (nh0,nh1) stays adjacent (shared psum
                    # group). The ns0-1 out-proj halves pop at slot >= 41,
                    # after head 3's normalize is emitted — the PE queue is
                    # in-order, so popping earlier would head-of-line block
                    # attention matmuls on the OT dep.
                    sched = list(front)                     # slots 1-4
                    for ii, st, ec in opc:                  # slots 5-40
                        sched.append(lambda ii=ii, st=st, ec=ec:
                                     outproj_chunk(ii, st, ec, 0))
                        sched.append(lambda ii=ii, st=st, ec=ec:
                                     outproj_chunk(ii, st, ec, 1))
                        sched.append(lambda: None)
                    for st in range(4):                     # slots 41-72
                        for ec in range(2):
                            sched.append(lambda st=st, ec=ec:
                                         outproj3_half(st, ec, lo=True))
                            sched += [lambda: None] * 3
                    fillers = sched
                else:
                    if opc:
                        mixed = list(front)
                        front = []
                        fi = iter(fillers)
                        for ii, st, ec in opc:
                            mixed.append(lambda ii=ii, st=st, ec=ec:
                                         outproj_chunk(ii, st, ec, 0))
                            mixed.append(lambda ii=ii, st=st, ec=ec:
                                         outproj_chunk(ii, st, ec, 1))
                            got = 0
                            for _ in range(2):
                                try:
                                    mixed.append(next(fi))
                                    got += 1
                                except StopIteration:
                                    break
                            if i >= 2 and got == 0:
                                # spread bare units so strip-2's late heads
                                # keep the PE fed (same fix as strip 3)
                                mixed.append(lambda: None)
                        mixed.extend(fi)
                        fillers = mixed
                    fillers = front + fillers
                fit = iter(fillers)
                attention(i, fit)
                for f in fit:   # leftover fillers
                    f()

            # ---- final strip out-projection tail: only the ns2-3 half
            # remains; evacs split across Scalar+Vector and psum groups
            # alternate between the (now idle) psA2 banks and ps_mm.
            # Throwaway matmuls first: they run during the last head's
            # normalize chain (~3.4us of otherwise-idle PE) so the DVFS
            # clock stays ramped for the 16 real tail matmuls. ----
            wps = psum.tile([128, STRIP], FP32, name="warm", tag="psA2",
                            bufs=2)
            for k in range(16):
                nc.tensor.matmul(wps[0:8, :], lhsT=wo_sb[:, 0, 0:8],
                                 rhs=wo_sb[:, 0, 0:STRIP],
                                 start=(k == 0), stop=(k == 15))
            for st in range(4):
                for ec in range(2):
                    outproj3_half(st, ec, lo=False,
                                  ptag="psA2" if ec == 0 else "ps_mm",
                                  evac="s" if ec == 0 else "v")
    nc.compile()
    return nc


_CACHE = {}


def _tri_mask():
    # T[p, l, c] = 1.0 if c >= p else 0 (keep sq >= sk on diagonal corners)
    p = np.arange(128)[:, None, None]
    c = np.arange(128)[None, None, :]
    return np.broadcast_to(
        (c >= p), (128, 2, 128)).astype(np.float32).astype(ml_dtypes.bfloat16)


def kernel(x, W_qkv, b_qkv, W_o, b_o):
    x = np.ascontiguousarray(np.asarray(x, dtype=np.float32))
    W_qkv = np.asarray(W_qkv, dtype=np.float32)
    b_qkv = np.asarray(b_qkv, dtype=np.float32)
    W_o = np.asarray(W_o, dtype=np.float32)
    b_o = np.asarray(b_o, dtype=np.float32)

    if "nc" not in _CACHE:
        _CACHE["nc"] = build_bass()
    nc = _CACHE["nc"]

    in_maps = []
    for c in range(N_CORES):
        b, g = c // G, c % G
        n0 = g * NG
        bq = b_qkv[n0:n0 + NG]
        bk = b_qkv[D + n0:D + n0 + NG]
        bqk = np.concatenate(
            [bq.reshape(4, 128).T, bk.reshape(4, 128).T], axis=1)  # [128, 8]
        BF = ml_dtypes.bfloat16

        def _w(m):  # [D, NG] -> [128, DS, NG] contiguous bf16
            return m.reshape(DS, 128, -1).transpose(1, 0, 2).astype(BF)

        def _wh(m):  # [D, NG] -> [4, 128, DS, 128] (contiguous per-nb) bf16
            r = m.reshape(DS, 128, 4, 128)
            return r.transpose(2, 1, 0, 3).astype(BF)
        in_maps.append({
            "xt": x[b].T.astype(BF),
            "wq": _wh(W_qkv[:, n0:n0 + NG]),
            "wk": _wh(W_qkv[:, D + n0:D + n0 + NG]),
            "wv": _w(W_qkv[:, 2 * D + n0:2 * D + n0 + NG]),
            "bqk": np.ascontiguousarray(bqk),
            "bv": np.ascontiguousarray(
                b_qkv[2 * D + n0:2 * D + n0 + NG].reshape(1, NG)),
            "wo": W_o[n0:n0 + NG, :].reshape(4, 128, D).transpose(1, 0, 2)
                     .astype(BF),
            "tri": _tri_mask(),
        })

    _CACHE["in_maps"] = in_maps
    res = run_bass_kernel_spmd(nc, in_maps, list(range(N_CORES)))
    outs = res.results

    out = np.empty((B, S, D), dtype=np.float32)
    for b in range(B):
        out[b] = (outs[G * b]["out"].astype(np.float32)
                  + outs[G * b + 1]["out"].astype(np.float32))
        out[b][(NSTRIP - 1) * STRIP:] += (
            outs[G * b]["out2"].astype(np.float32)
            + outs[G * b + 1]["out2"].astype(np.float32))
    out += b_o[None, None, :]
    return out

